# revision 1
# baseline (speedup 1.0000x reference)
"""GQA attention kernel for 8 TRN2 NeuronCores.

Sharding: core c = (batch b = c//4, kv-head h = c%4). Each core computes its
batch's projections for its KV head + the 4 query heads of that group, runs
causal attention in S^T layout (softmax reduction folded into the PV matmul
via an appended ones-column on V), and produces a partial output for its
256 columns of Wo. Host sums the 4 partials per batch.

All matmuls run as float32r (1 cycle/row on the PE vs 4 for fp32,
~1.5e-4 relative rounding).
"""
import sys, os
sys.path.insert(0, "/opt/trn_rl_repo")
os.environ.setdefault("MYCRO_LOCAL_CACHE", "1")

import numpy as np
from contextlib import ExitStack

import concourse.bass as bass
import concourse.tile as tile
from concourse import bacc, mybir
from concourse.bass_utils import run_bass_kernel_spmd

F32, F32R = mybir.dt.float32, mybir.dt.float32r
AF = mybir.ActivationFunctionType

B, S, DM = 2, 2048, 1024
H, HKV, DK = 16, 4, 64
G = H // HKV                 # 4 query heads per core
NKT = DM // 128              # 8 dmodel k-tiles
NSQ = S // 512               # 4 sq tiles
NSK = S // 128               # 16 sk tiles
N_CORES = 8

_nc_cache = None


def _build():
    nc = bacc.Bacc("TRN2", target_bir_lowering=False, debug=False)
    inp = {}
    for name, shape in [
        ("xqT", [DM, S]), ("xkT", [DM, S]), ("xvT", [DM, S]),
        ("wqT", [DM, G * DK]), ("wkT", [DM, DK]), ("wvT", [DM, DK]),
        ("woT", [G * DK, DM]),
        ("cos2", [128, S]), ("sin2", [128, S]),
        ("r2T", [128, 128]), ("ident", [64, 64]),
        ("masks", [128, 4 * 512]),
    ]:
        inp[name] = nc.dram_tensor(name, shape, F32, kind="ExternalInput").ap()
    out = nc.dram_tensor("out", [S, DM], F32, kind="ExternalOutput").ap()

    with tile.TileContext(nc) as tc, ExitStack() as ctx:
        const = ctx.enter_context(tc.tile_pool(name="const", bufs=1))
        sb = ctx.enter_context(tc.tile_pool(name="sb", bufs=2))
        sbx = ctx.enter_context(tc.tile_pool(name="sbx", bufs=8))
        ps = ctx.enter_context(tc.tile_pool(name="ps", bufs=3, space="PSUM"))
        ps_acc = ctx.enter_context(tc.tile_pool(name="ps_acc", bufs=2, space="PSUM"))
        ps_tr = ctx.enter_context(tc.tile_pool(name="ps_tr", bufs=2, space="PSUM"))

        def load_const(name, shape, dtype=F32R, eng=None):
            if dtype == F32:
                t = const.tile(shape, F32, tag=name + "_raw")
                nc.sync.dma_start(t[:], inp[name][:])
                return t
            r = const.tile(shape, F32R, tag=name)
            nc.gpsimd.dma_start(r[:], inp[name][:])
            return r

        # weights: DRAM [DM, M] -> SBUF [128, NKT*M] (k-tiles along free dim)
        def load_wT(name, m):
            r = const.tile([128, NKT * m], F32R, tag=name)
            for kt in range(NKT):
                nc.gpsimd.dma_start(r[:, kt * m:(kt + 1) * m],
                                    inp[name][kt * 128:(kt + 1) * 128, :])
            return r

        wq_sb = load_wT("wqT", G * DK)        # [128, 8*256]
        wk_sb = load_wT("wkT", DK)            # [128, 8*64]
        wv_sb = load_wT("wvT", DK)
        wo_sb = const.tile([128, 2 * DM], F32R, tag="wo_sb")
        nc.gpsimd.dma_start(wo_sb[:, 0:DM], inp["woT"][0:128, :])
        nc.gpsimd.dma_start(wo_sb[:, DM:2 * DM], inp["woT"][128:256, :])
        cos_sb = load_const("cos2", [128, S], F32)
        sin_sb = load_const("sin2", [128, S], F32)
        r2_sb = load_const("r2T", [128, 128])
        id_sb = load_const("ident", [64, 64])
        mask_sb = load_const("masks", [128, 4 * 512], F32)

        # persistent activations
        qt = [const.tile([128, S], F32R, tag=f"qt{i}", name=f"qt{i}") for i in range(2)]
        krope = const.tile([64, S], F32R, tag="krope")
        khi = const.tile([128, S], F32R, tag="khi")
        v_sb = const.tile([128, NSK, 65], F32R, tag="v_sb")
        ot = [const.tile([128, S], F32R, tag=f"ot{i}", name=f"ot{i}") for i in range(2)]

        def x_chunk(name, kt, st):
            r = sbx.tile([128, 512], F32R, tag=name + "_r")
            nc.gpsimd.dma_start(r[:],
                                inp[name][kt * 128:(kt + 1) * 128, st * 512:(st + 1) * 512])
            return r

        # ---- Q projection + rope (heads packed 2+2 into qt[0], qt[1])
        for st in range(NSQ):
            xq = [x_chunk("xqT", kt, st) for kt in range(NKT)]
            for half in range(2):
                psQ = ps.tile([128, 512], F32, tag="big")
                for kt in range(NKT):
                    o = kt * G * DK + half * 128
                    nc.tensor.matmul(psQ[:], wq_sb[:, o:o + 128], xq[kt][:],
                                     start=(kt == 0), stop=(kt == NKT - 1))
                qsb = sb.tile([128, 512], F32R, tag="pcopy")
                nc.vector.tensor_copy(qsb[:], psQ[:])
                psRot = ps.tile([128, 512], F32, tag="big")
                nc.tensor.matmul(psRot[:], r2_sb[:], qsb[:], start=True, stop=True)
                t1 = sb.tile([128, 512], F32, tag="t1")
                nc.vector.tensor_mul(t1[:], qsb[:], cos_sb[:, st * 512:(st + 1) * 512])
                t2 = sb.tile([128, 512], F32, tag="t2")
                nc.vector.tensor_mul(t2[:], psRot[:], sin_sb[:, st * 512:(st + 1) * 512])
                nc.vector.tensor_add(qt[half][:, st * 512:(st + 1) * 512], t1[:], t2[:])

        # ---- K + V projections
        for st in range(NSQ):
            xk = [x_chunk("xkT", kt, st) for kt in range(NKT)]
            xv = [x_chunk("xvT", kt, st) for kt in range(NKT)]
            psK = ps.tile([64, 512], F32, tag="big")
            for kt in range(NKT):
                nc.tensor.matmul(psK[:], wk_sb[:, kt * DK:(kt + 1) * DK], xk[kt][:],
                                 start=(kt == 0), stop=(kt == NKT - 1))
            ksb = sb.tile([64, 512], F32R, tag="pcopy")
            nc.vector.tensor_copy(ksb[:], psK[:])
            psRotK = ps.tile([64, 512], F32, tag="big")
            nc.tensor.matmul(psRotK[:], r2_sb[0:64, 0:64], ksb[:], start=True, stop=True)
            k1 = sb.tile([64, 512], F32, tag="t1")
            nc.vector.tensor_mul(k1[:], ksb[:], cos_sb[0:64, st * 512:(st + 1) * 512])
            k2 = sb.tile([64, 512], F32, tag="t2")
            nc.vector.tensor_mul(k2[:], psRotK[:], sin_sb[0:64, st * 512:(st + 1) * 512])
            nc.vector.tensor_add(krope[:, st * 512:(st + 1) * 512], k1[:], k2[:])
            nc.sync.dma_start(khi[64:128, st * 512:(st + 1) * 512],
                              krope[:, st * 512:(st + 1) * 512])

            psVT = ps.tile([64, 512], F32, tag="big")
            for kt in range(NKT):
                nc.tensor.matmul(psVT[:], wv_sb[:, kt * DK:(kt + 1) * DK], xv[kt][:],
                                 start=(kt == 0), stop=(kt == NKT - 1))
            vtsb = sb.tile([64, 512], F32R, tag="pcopy")
            nc.vector.tensor_copy(vtsb[:], psVT[:])
            for j in range(4):
                psVtr = ps_tr.tile([128, 64], F32R, tag="tr")
                nc.tensor.transpose(psVtr[:], vtsb[:, j * 128:(j + 1) * 128], id_sb[:])
                nc.vector.tensor_copy(v_sb[:, st * 4 + j, 0:64], psVtr[:])
        nc.gpsimd.memset(v_sb[:, :, 64:65].bitcast(F32), 1.0)

        # ---- attention: h in 4 query heads, st in 4 sq tiles (causal sk range)
        for h in range(G):
            half, sub = h // 2, h % 2
            for st in range(NSQ):
                psO = ps_acc.tile([65, 512], F32, tag="acc")
                nsk = 4 * st + 4
                for skt in range(nsk):
                    di = skt - 4 * st            # >=0 on diagonal tiles
                    psS = ps.tile([128, 512], F32, tag="big")
                    if sub == 0:
                        lhsT = krope[:, skt * 128:(skt + 1) * 128]
                        rhs = qt[half][0:64, st * 512:(st + 1) * 512]
                    else:
                        lhsT = khi[64:128, skt * 128:(skt + 1) * 128]
                        rhs = qt[half][64:128, st * 512:(st + 1) * 512]
                    nc.tensor.matmul(psS[:], lhsT, rhs, start=True, stop=True)
                    pt2 = sb.tile([128, 512], F32R, tag="pt2")
                    if di >= 0:
                        pt = sb.tile([128, 512], F32, tag="pt")
                        nc.scalar.activation(pt[:], psS[:], AF.Exp)
                        nc.vector.tensor_mul(pt2[:], pt[:],
                                             mask_sb[:, di * 512:(di + 1) * 512])
                    else:
                        nc.scalar.activation(pt2[:], psS[:], AF.Exp)
                    nc.tensor.matmul(psO[:], v_sb[:, skt, :], pt2[:],
                                     start=(skt == 0), stop=(skt == nsk - 1))
                recip = sb.tile([128, 512], F32, tag="recip")
                nc.vector.reciprocal(recip[64:65, :], psO[64:65, :])
                recip0 = sb.tile([1, 512], F32, tag="recip0")
                nc.sync.dma_start(recip0[:], recip[64:65, :])
                bcast = sb.tile([64, 512], F32, tag="bcast")
                nc.gpsimd.partition_broadcast(bcast[:], recip0[:])
                if sub == 0:
                    nc.vector.tensor_mul(ot[half][0:64, st * 512:(st + 1) * 512],
                                         psO[0:64, :], bcast[:])
                else:
                    tmp = sb.tile([64, 512], F32R, tag="otmp")
                    nc.vector.tensor_mul(tmp[:], psO[0:64, :], bcast[:])
                    nc.sync.dma_start(ot[half][64:128, st * 512:(st + 1) * 512], tmp[:])

        # ---- output projection
        for st in range(S // 128):
            for dt in range(2):
                psF = ps.tile([128, 512], F32, tag="big")
                nc.tensor.matmul(psF[:], ot[0][:, st * 128:(st + 1) * 128],
                                 wo_sb[:, dt * 512:(dt + 1) * 512],
                                 start=True, stop=False)
                nc.tensor.matmul(psF[:], ot[1][:, st * 128:(st + 1) * 128],
                                 wo_sb[:, DM + dt * 512:DM + (dt + 1) * 512],
                                 start=False, stop=True)
                osb = sb.tile([128, 512], F32, tag="osb")
                nc.scalar.copy(osb[:], psF[:])
                nc.sync.dma_start(out[st * 128:(st + 1) * 128,
                                      dt * 512:(dt + 1) * 512], osb[:])

    nc.compile()
    return nc


def _host_inputs(query, key, value, Wq, Wk, Wv, Wo):
    inv_freq = 1.0 / (10000.0 ** (np.arange(0, DK, 2, dtype=np.float64) / DK))
    t = np.arange(S, dtype=np.float64)
    freqs = np.einsum("s,f->sf", t, inv_freq)
    emb = np.concatenate([freqs, freqs], axis=-1)
    cos = np.cos(emb).astype(np.float32).T.copy()   # [64, S]
    sin = np.sin(emb).astype(np.float32).T.copy()
    cos2 = np.concatenate([cos, cos], axis=0).copy()
    sin2 = np.concatenate([sin, sin], axis=0).copy()
    R = np.zeros((DK, DK), np.float32)
    half = DK // 2
    for d in range(half):
        R[d, d + half] = -1.0
        R[d + half, d] = 1.0
    r2T = np.zeros((128, 128), np.float32)
    r2T[0:64, 0:64] = R.T
    r2T[64:128, 64:128] = R.T
    ident = np.eye(64, dtype=np.float32)
    masks = np.zeros((128, 4 * 512), np.float32)
    rr = np.arange(128)[:, None]
    cc = np.arange(512)[None, :]
    for i in range(4):
        masks[:, i * 512:(i + 1) * 512] = (rr <= cc - 128 * i).astype(np.float32)

    in_maps = []
    for c in range(N_CORES):
        b, h = c // HKV, c % HKV
        in_maps.append({
            "xqT": np.ascontiguousarray(query[b].T),
            "xkT": np.ascontiguousarray(key[b].T),
            "xvT": np.ascontiguousarray(value[b].T),
            "wqT": np.ascontiguousarray((Wq[h * G * DK:(h + 1) * G * DK, :] * 0.125).T),
            "wkT": np.ascontiguousarray(Wk[h * DK:(h + 1) * DK, :].T),
            "wvT": np.ascontiguousarray(Wv[h * DK:(h + 1) * DK, :].T),
            "woT": np.ascontiguousarray(Wo[:, h * G * DK:(h + 1) * G * DK].T),
            "cos2": cos2, "sin2": sin2, "r2T": r2T, "ident": ident, "masks": masks,
        })
    return in_maps


def kernel(query, key, value, Wq, Wk, Wv, Wo):
    global _nc_cache
    query, key, value = (np.asarray(a, np.float32) for a in (query, key, value))
    Wq, Wk, Wv, Wo = (np.asarray(a, np.float32) for a in (Wq, Wk, Wv, Wo))
    in_maps = _host_inputs(query, key, value, Wq, Wk, Wv, Wo)
    if _nc_cache is None:
        _nc_cache = _build()
    res = run_bass_kernel_spmd(_nc_cache, in_maps, list(range(N_CORES)))
    out = np.zeros((B, S, DM), np.float32)
    for c in range(N_CORES):
        out[c // HKV] += res.results[c]["out"]
    return out

